# revision 8
# baseline (speedup 1.0000x reference)
"""ConcatCritic pair-MLP kernel for 8 Trainium2 NeuronCores.

scores[i, j] = MLP(concat(x_i, y_j)) with
MLP = Linear(256,512) -> ReLU -> Linear(512,512) -> ReLU -> Linear(512,1).

Sharding: pure data parallelism over the B^2 pair grid, split along the
x (row) index — each core gets 64 rows of x plus all of y and the full
(small) weight set, and produces a [64, 512] slab of the output.

The host passes x and y pre-transposed (xT [128,64] slab, yT [128,512]) and
receives the output in transposed chunk layout [4][128 j][64 i]; both
transposes are trivial numpy work and remove every on-device transpose.

All matmul operands are fp16 (host-converted): the PE runs fp16 at the same
1 cycle/row as fp32r, but fp16 stationaries re-enable the compiler's fast
weight load (FWL — disabled for fp32/fp32r), hiding the per-matmul LDWEIGHTS
that cost the fp32r version ~22ns/matmul, and input DMA bytes halve.
Accumulation stays fp32 in PSUM; stage C (relu * W3 + accumulate) runs in
fp32 on the DVE, so the only precision loss is fp16 operand rounding
(measured 5e-4 rel-to-max vs the fp32 reference, gate is 2e-2).

Per-core dataflow (per x-row i):
  stage A (ACT): h1T[h, j] = relu(hyT[h, j] + (hx_i[h] + b1[h]))     4x [128,512]
  stage B (PE):  x2[j, k]  = h1_i @ W2   (fp16 matmuls, psum accum)  16x mm
  stage C (DVE): s_i[j]    = sum_k relu(x2[j, k]) * W3[k]            4x scalar_tensor_tensor
with hxT = (x @ W1[:128]).T + b1 and hyT = (y @ W1[128:]).T computed once at
setup. b3 is applied on the host; b2 (nonzero only) via an extra K=1 matmul.
"""

import numpy as np

B = 512
DX = 128
H = 512
N_CORES = 8
ROWS = B // N_CORES  # 64 x-rows per core
HC = H // 128  # 4 chunks of the hidden dim

_BUILT = {}  # with_b2 -> bass.Bass


def _build(with_b2: bool):
    import concourse.mybir as mybir
    from concourse.bacc import Bacc
    from concourse.tile import TileContext

    F32 = mybir.dt.float32
    F16 = mybir.dt.float16
    Relu = mybir.ActivationFunctionType.Relu
    Alu = mybir.AluOpType

    # Bacc (not raw Bass): its compile pipeline splits multi-semaphore waits
    # into event-semaphore chains — TRN2 engine instructions accept only one
    # sync wait, which walrus otherwise rejects.
    nc = Bacc()
    xT_d = nc.declare_dram_parameter("xT", [DX, ROWS], F16, isOutput=False)
    yT_d = nc.declare_dram_parameter("yT", [DX, B], F16, isOutput=False)
    w1_d = nc.declare_dram_parameter("W1", [2 * DX, H], F16, isOutput=False)
    b1_d = nc.declare_dram_parameter("b1", [H], F32, isOutput=False)
    w2_d = nc.declare_dram_parameter("W2", [H, H], F16, isOutput=False)
    w3_d = nc.declare_dram_parameter("W3", [H, 1], F32, isOutput=False)
    if with_b2:
        b2_d = nc.declare_dram_parameter("b2", [H], F16, isOutput=False)
    out_d = nc.declare_dram_parameter("outT", [HC, 128, ROWS], F32, isOutput=True)

    with TileContext(nc) as tc:
        with (
            tc.tile_pool(name="consts", bufs=1) as cpool,
            tc.tile_pool(name="work", bufs=2) as wpool,
            tc.tile_pool(name="psum", bufs=7, space="PSUM") as ppool,
        ):
            # ---------------- input DMAs ----------------
            # Only the two HWDGE rings (sync/SP and scalar/ACT) — the gpsimd
            # SWDGE ring has a slow software drain in the epilogue. The DMA
            # critical path gates hy/hx, which gate everything: tiny tensors
            # (b1, xT) go first for latency, then yT/w1y/w1x each SPLIT in
            # half across both rings so both land ~0.5us earlier, then the W2
            # chunks (first needed ~1.5us later), then w3row (stage C only).
            yT = cpool.tile([DX, B], F16, name="yT")
            w1y = cpool.tile([DX, H], F16, name="w1y")
            w1x = cpool.tile([DX, H], F16, name="w1x")
            b1sb = cpool.tile([128, HC], F32, name="b1sb")  # [p, hc]
            xT = cpool.tile([DX, ROWS], F16, name="xT")
            HB = B // 2
            nc.scalar.dma_start(out=b1sb[:], in_=b1_d[:].rearrange("(c p) -> p c", p=128))
            nc.scalar.dma_start(out=xT[:], in_=xT_d[:, :])
            nc.sync.dma_start(out=yT[:, 0:HB], in_=yT_d[:, 0:HB])
            nc.scalar.dma_start(out=yT[:, HB:], in_=yT_d[:, HB:])
            nc.sync.dma_start(out=w1y[:, 0:HB], in_=w1_d[DX : 2 * DX, 0:HB])
            nc.scalar.dma_start(out=w1y[:, HB:], in_=w1_d[DX : 2 * DX, HB:])
            nc.sync.dma_start(out=w1x[:, 0:HB], in_=w1_d[0:DX, 0:HB])
            nc.scalar.dma_start(out=w1x[:, HB:], in_=w1_d[0:DX, HB:])
            # W2 chunks 2/3 ride the third (gpsimd SWDGE) queue — idle at
            # startup — so all 512KB of W2 lands ~1us sooner than two-queue.
            w2sb = [cpool.tile([128, H], F16, name=f"w2_{hc}") for hc in range(HC)]
            w2r = w2_d[:, :].rearrange("(c p) k -> p c k", p=128)
            nc.gpsimd.dma_start(out=w2sb[2][:], in_=w2r[:, 2, :])
            nc.gpsimd.dma_start(out=w2sb[3][:], in_=w2r[:, 3, :])
            nc.sync.dma_start(out=w2sb[0][:], in_=w2r[:, 0, :])
            nc.scalar.dma_start(out=w2sb[1][:], in_=w2r[:, 1, :])
            w3row = cpool.tile([1, H], F32, name="w3row")
            nc.scalar.dma_start(out=w3row[:], in_=w3_d[:, :].flatten().unsqueeze(0))
            if with_b2:
                b2row = cpool.tile([1, H], F16, name="b2row")
                nc.scalar.dma_start(out=b2row[:], in_=b2_d[:].unsqueeze(0))
                ones1 = cpool.tile([1, 128], F16, name="ones1")
                nc.vector.memset(ones1[:], 1.0)

            # PE warmup: the HAM clock gate holds the PE at 1.2GHz until
            # ~3.4us of sustained GAPLESS activity; any idle gap resets the
            # timer. Burn dummy matmuls into a scratch psum bank while the
            # input DMAs are in flight, sized to end right as yT/w1y land.
            # warm_src memset on the DVE (idle at startup; gpsimd issues it
            # ~1us later) so the PE starts — and so the ramp completes —
            # earlier.
            warm_src = cpool.tile([1, 128], F16, name="warm_src")
            nc.vector.memset(warm_src[:], 1.0)
            pswarm = ppool.tile([128, 64], F32, name="pswarm", tag="warm", bufs=1)
            for _ in range(56):
                nc.tensor.matmul(
                    pswarm[:], warm_src[:], warm_src[:, 0:64], start=True, stop=True
                )

            # hy/hx interleaved so hyT[0]+hxT[0] (the row-0 stage-A inputs)
            # are ready as early as possible.
            # hyT[hc][h, j] = (y @ W1y).T stored fp16; hxT[hc][h, i] =
            # (x @ W1x).T + b1 kept fp32 (per-partition bias).
            hyT = []
            hxT = []
            for hc in range(HC):
                pshy = ppool.tile([128, B], F32, name="pshy", tag="ps")
                nc.tensor.matmul(
                    pshy[:], w1y[:, hc * 128 : (hc + 1) * 128], yT[:], start=True, stop=True
                )
                ty = cpool.tile([128, B], F16, name=f"hyT_{hc}")
                nc.vector.tensor_copy(out=ty[:], in_=pshy[:])
                hyT.append(ty)
                pshx = ppool.tile([128, ROWS], F32, name="pshx", tag="ps")
                nc.tensor.matmul(
                    pshx[:], w1x[:, hc * 128 : (hc + 1) * 128], xT[:], start=True, stop=True
                )
                tx = cpool.tile([128, ROWS], F32, name=f"hxT_{hc}")
                nc.vector.tensor_scalar_add(tx[:], pshx[:], b1sb[:, hc : hc + 1])
                hxT.append(tx)

            # W3 broadcast to all 128 partitions via a K=1 ones-matmul
            # (gpsimd partition_broadcast would wake the SWDGE ring; a
            # stride-0 DMA would re-read the 2KB row 128 times from HBM).
            ones_col = cpool.tile([1, 128], F32, name="ones_col")
            nc.gpsimd.memset(ones_col[:], 1.0)
            psb = ppool.tile([128, H], F32, name="psb", tag="ps")
            nc.tensor.matmul(psb[:], ones_col[:], w3row[:], start=True, stop=True)
            w3b = cpool.tile([128, H], F32, name="w3b")
            nc.vector.tensor_copy(out=w3b[:], in_=psb[:])

            # PE keepalive over the row-0 stage-A bubble (ACT needs ~700ns
            # before the first real layer-2 matmul can start): keep the HAM
            # activity timer running so the clock ramp isn't reset.
            for _ in range(14):
                nc.tensor.matmul(
                    pswarm[:], warm_src[:], warm_src[:, 0:64], start=True, stop=True
                )

            # scores accumulated transposed: scoresT[jc][j, i]
            scoresT = [cpool.tile([128, ROWS], F32, name=f"scT_{jc}") for jc in range(HC)]

            # ---------------- main loop over x rows ----------------
            for i in range(ROWS):
                h1T = []
                for hc in range(HC):
                    # ACT: relu(hyT + hx_i). All of stage A lives on ACT so the
                    # DVE has headroom for the stage-C fused reduce.
                    t = wpool.tile([128, B], F16, name="h1T", tag="h1T", bufs=12)
                    nc.scalar.activation(
                        t[:], hyT[hc][:], Relu, bias=hxT[hc][:, i : i + 1], scale=1.0
                    )
                    h1T.append(t)
                for jc in range(HC):
                    ps2 = ppool.tile([128, B], F32, name="ps2", tag="ps")
                    for hc in range(HC):
                        nc.tensor.matmul(
                            ps2[:],
                            h1T[hc][:, jc * 128 : (jc + 1) * 128],
                            w2sb[hc][:],
                            start=(hc == 0),
                            stop=(hc == HC - 1 and not with_b2),
                        )
                    if with_b2:
                        nc.tensor.matmul(
                            ps2[:], ones1[:], b2row[:], start=False, stop=True
                        )
                    # DVE: scr = relu(ps2) * W3_bcast; scoresT col = sum_k scr
                    scr = wpool.tile([128, B], F32, name="scr", tag="scr", bufs=6)
                    nc.vector.scalar_tensor_tensor(
                        out=scr[:],
                        in0=ps2[:],
                        scalar=0.0,
                        in1=w3b[:],
                        op0=Alu.max,
                        op1=Alu.mult,
                        accum_out=scoresT[jc][:, i : i + 1],
                    )

                # stream the output: every 16 rows, DMA the finished 16-col
                # slab of each scoresT chunk (8KB apiece), so only the last
                # slab remains after the final matmul. Sync ring ONLY: a
                # mid-loop DMA on the scalar ring blocks the ACT engine's
                # strict-FIFO queue on the stt semaphore and starves stage A
                # (measured +43ns on every matmul).
                if i % 16 == 15:
                    lo, hi = i - 15, i + 1
                    for jc in range(HC):
                        nc.sync.dma_start(
                            out=out_d[jc, :, lo:hi], in_=scoresT[jc][:, lo:hi]
                        )

    nc.finalize()  # runs the Bacc pass pipeline (wait splitting etc.)
    return nc


def _get_nc(with_b2: bool):
    if with_b2 not in _BUILT:
        _BUILT[with_b2] = _build(with_b2)
    return _BUILT[with_b2]


def _run(inputs: dict, trace: bool = False, **spmd_kwargs):
    """Shard, execute on 8 cores, gather. Returns (scores, BassKernelResults)."""
    from concourse.bass_utils import run_bass_kernel_spmd

    x = np.asarray(inputs["x"], dtype=np.float32)
    y = np.asarray(inputs["y"], dtype=np.float32)
    W1 = np.asarray(inputs["W1"], dtype=np.float32)
    b1 = np.ascontiguousarray(np.asarray(inputs["b1"], dtype=np.float32))
    W2 = np.asarray(inputs["W2"], dtype=np.float32)
    b2 = np.ascontiguousarray(np.asarray(inputs.get("b2", np.zeros(H)), dtype=np.float32))
    W3 = np.ascontiguousarray(np.asarray(inputs["W3"], dtype=np.float32).reshape(H, 1))
    b3 = np.asarray(inputs.get("b3", np.zeros(1)), dtype=np.float32)

    with_b2 = bool(np.any(b2))
    nc = _get_nc(with_b2)

    W1h = np.ascontiguousarray(W1.astype(np.float16))
    W2h = np.ascontiguousarray(W2.astype(np.float16))
    yTh = np.ascontiguousarray(y.T.astype(np.float16))
    xh = x.astype(np.float16)
    in_maps = []
    for c in range(N_CORES):
        m = {
            "xT": np.ascontiguousarray(xh[c * ROWS : (c + 1) * ROWS].T),
            "yT": yTh,
            "W1": W1h,
            "b1": b1,
            "W2": W2h,
            "W3": W3,
        }
        if with_b2:
            m["b2"] = np.ascontiguousarray(b2.astype(np.float16))
        in_maps.append(m)

    res = run_bass_kernel_spmd(
        nc, in_maps, core_ids=list(range(N_CORES)), trace=trace, **spmd_kwargs
    )
    # outT[jc, j, i] -> scores_slab[i, jc*128 + j]
    slabs = [
        np.transpose(r["outT"], (2, 0, 1)).reshape(ROWS, B) for r in res.results
    ]
    out = np.concatenate(slabs, axis=0)
    if b3.size and np.any(b3):
        out = out + b3.reshape(-1)[0]
    return np.ascontiguousarray(out.astype(np.float32)), res


def kernel(**inputs) -> np.ndarray:
    out, _ = _run(inputs)
    return out



# revision 11
# speedup vs baseline: 1.0014x; 1.0014x over previous
"""ConcatCritic pair-MLP kernel for 8 Trainium2 NeuronCores.

scores[i, j] = MLP(concat(x_i, y_j)) with
MLP = Linear(256,512) -> ReLU -> Linear(512,512) -> ReLU -> Linear(512,1).

Sharding: pure data parallelism over the B^2 pair grid, split along the
x (row) index — each core gets 64 rows of x plus all of y and the full
(small) weight set, and produces a [64, 512] slab of the output.

The host passes x and y pre-transposed (xT [128,64] slab, yT [128,512]) and
receives the output in transposed chunk layout [4][128 j][64 i]; both
transposes are trivial numpy work and remove every on-device transpose.

All matmul operands are fp16 (host-converted): the PE runs fp16 at the same
1 cycle/row as fp32r, but fp16 stationaries re-enable the compiler's fast
weight load (FWL — disabled for fp32/fp32r), hiding the per-matmul LDWEIGHTS
that cost the fp32r version ~22ns/matmul, and input DMA bytes halve.
Accumulation stays fp32 in PSUM; stage C (relu * W3 + accumulate) runs in
fp32 on the DVE, so the only precision loss is fp16 operand rounding
(measured 5e-4 rel-to-max vs the fp32 reference, gate is 2e-2).

Per-core dataflow (per x-row i):
  stage A (ACT): h1T[h, j] = relu(hyT[h, j] + (hx_i[h] + b1[h]))     4x [128,512]
  stage B (PE):  x2[j, k]  = h1_i @ W2   (fp16 matmuls, psum accum)  16x mm
  stage C (DVE): s_i[j]    = sum_k relu(x2[j, k]) * W3[k]            4x scalar_tensor_tensor
with hxT = (x @ W1[:128]).T + b1 and hyT = (y @ W1[128:]).T computed once at
setup. b3 is applied on the host; b2 (nonzero only) via an extra K=1 matmul.
"""

import numpy as np

B = 512
DX = 128
H = 512
N_CORES = 8
ROWS = B // N_CORES  # 64 x-rows per core
HC = H // 128  # 4 chunks of the hidden dim

_BUILT = {}  # with_b2 -> bass.Bass


def _build(with_b2: bool):
    import concourse.mybir as mybir
    from concourse.bacc import Bacc
    from concourse.tile import TileContext

    F32 = mybir.dt.float32
    F16 = mybir.dt.float16
    Relu = mybir.ActivationFunctionType.Relu
    Alu = mybir.AluOpType

    # Bacc (not raw Bass): its compile pipeline splits multi-semaphore waits
    # into event-semaphore chains — TRN2 engine instructions accept only one
    # sync wait, which walrus otherwise rejects.
    nc = Bacc()
    xT_d = nc.declare_dram_parameter("xT", [DX, ROWS], F16, isOutput=False)
    yT_d = nc.declare_dram_parameter("yT", [DX, B], F16, isOutput=False)
    w1_d = nc.declare_dram_parameter("W1", [2 * DX, H], F16, isOutput=False)
    b1_d = nc.declare_dram_parameter("b1", [H], F32, isOutput=False)
    w2_d = nc.declare_dram_parameter("W2", [H, H], F16, isOutput=False)
    w3_d = nc.declare_dram_parameter("W3", [H, 1], F32, isOutput=False)
    if with_b2:
        b2_d = nc.declare_dram_parameter("b2", [H], F16, isOutput=False)
    out_d = nc.declare_dram_parameter("outT", [HC, 128, ROWS], F32, isOutput=True)

    with TileContext(nc) as tc:
        with (
            tc.tile_pool(name="consts", bufs=1) as cpool,
            tc.tile_pool(name="work", bufs=2) as wpool,
            tc.tile_pool(name="psum", bufs=7, space="PSUM") as ppool,
        ):
            # ---------------- input DMAs ----------------
            # Only the two HWDGE rings (sync/SP and scalar/ACT) — the gpsimd
            # SWDGE ring has a slow software drain in the epilogue. The DMA
            # critical path gates hy/hx, which gate everything: tiny tensors
            # (b1, xT) go first for latency, then yT/w1y/w1x each SPLIT in
            # half across both rings so both land ~0.5us earlier, then the W2
            # chunks (first needed ~1.5us later), then w3row (stage C only).
            yT = cpool.tile([DX, B], F16, name="yT")
            w1y = cpool.tile([DX, H], F16, name="w1y")
            w1x = cpool.tile([DX, H], F16, name="w1x")
            b1sb = cpool.tile([128, HC], F32, name="b1sb")  # [p, hc]
            xT = cpool.tile([DX, ROWS], F16, name="xT")
            HB = B // 2
            w3row = cpool.tile([1, H], F32, name="w3row")
            # sync ring: b1+xT (tiny, latency-bound), then the m0 halves of
            # the layer-1 tensors, then W2c0. scalar ring: w3row first (2KB,
            # gates w3b -> the first stts -> psum recycling), then the m1
            # halves, then W2c1. gpsimd SWDGE ring (idle at startup): W2c2/3.
            nc.sync.dma_start(out=b1sb[:], in_=b1_d[:].rearrange("(c p) -> p c", p=128))
            nc.sync.dma_start(out=xT[:], in_=xT_d[:, :])
            nc.scalar.dma_start(out=w3row[:], in_=w3_d[:, :].flatten().unsqueeze(0))
            nc.sync.dma_start(out=yT[:, 0:HB], in_=yT_d[:, 0:HB])
            nc.scalar.dma_start(out=yT[:, HB:], in_=yT_d[:, HB:])
            nc.sync.dma_start(out=w1y[:, 0:HB], in_=w1_d[DX : 2 * DX, 0:HB])
            nc.scalar.dma_start(out=w1y[:, HB:], in_=w1_d[DX : 2 * DX, HB:])
            nc.sync.dma_start(out=w1x[:, 0:HB], in_=w1_d[0:DX, 0:HB])
            nc.scalar.dma_start(out=w1x[:, HB:], in_=w1_d[0:DX, HB:])
            w2sb = [cpool.tile([128, H], F16, name=f"w2_{hc}") for hc in range(HC)]
            w2r = w2_d[:, :].rearrange("(c p) k -> p c k", p=128)
            nc.gpsimd.dma_start(out=w2sb[2][:], in_=w2r[:, 2, :])
            nc.gpsimd.dma_start(out=w2sb[3][:], in_=w2r[:, 3, :])
            nc.sync.dma_start(out=w2sb[0][:], in_=w2r[:, 0, :])
            nc.scalar.dma_start(out=w2sb[1][:], in_=w2r[:, 1, :])
            if with_b2:
                b2row = cpool.tile([1, H], F16, name="b2row")
                nc.scalar.dma_start(out=b2row[:], in_=b2_d[:].unsqueeze(0))
                ones1 = cpool.tile([1, 128], F16, name="ones1")
                nc.vector.memset(ones1[:], 1.0)

            # PE warmup: the HAM clock gate holds the PE at ~1.2GHz until
            # several us of sustained FULL-INTENSITY activity. Full-width
            # (N=512) dummy matmuls look exactly like main-loop work to the
            # activity monitor; tiny N=64 ones ramp it far more slowly.
            # 8 x ~427ns (cold) bridges until the layer-1 DMAs land.
            warm_src = cpool.tile([1, 128], F16, name="warm_src")
            nc.vector.memset(warm_src[:], 1.0)
            warm_mov = cpool.tile([1, B], F16, name="warm_mov")
            nc.vector.memset(warm_mov[:], 1.0)
            pswarm = ppool.tile([128, B], F32, name="pswarm", tag="warm", bufs=1)
            for _ in range(8):
                nc.tensor.matmul(
                    pswarm[:], warm_src[:], warm_mov[:], start=True, stop=True
                )

            # hy in half-width (N=256) matmuls against the yT halves, and
            # interleaved with hx, so PE work is gated on whichever DMA ring
            # happens to land first rather than on all of yT+w1y; hc 0/1
            # stationaries live in the m0 halves (sync ring), hc 2/3 in m1
            # (scalar ring). hyT[hc][h, j] = (y @ W1y).T stored fp16;
            # hxT[hc][h, i] = (x @ W1x).T + b1 kept fp32.
            hyT = [cpool.tile([128, B], F16, name=f"hyT_{hc}") for hc in range(HC)]
            hxT = []
            pshy = [None] * HC

            def hy_half(hc, m):
                lo, hi = m * HB, (m + 1) * HB
                if pshy[hc] is None:
                    pshy[hc] = ppool.tile([128, B], F32, name="pshy", tag="ps")
                nc.tensor.matmul(
                    pshy[hc][:, lo:hi],
                    w1y[:, hc * 128 : (hc + 1) * 128],
                    yT[:, lo:hi],
                    start=True,
                    stop=True,
                )
                nc.vector.tensor_copy(out=hyT[hc][:, lo:hi], in_=pshy[hc][:, lo:hi])

            def hx_one(hc):
                pshx = ppool.tile([128, ROWS], F32, name="pshx", tag="ps")
                nc.tensor.matmul(
                    pshx[:], w1x[:, hc * 128 : (hc + 1) * 128], xT[:], start=True, stop=True
                )
                tx = cpool.tile([128, ROWS], F32, name=f"hxT_{hc}")
                nc.vector.tensor_scalar_add(tx[:], pshx[:], b1sb[:, hc : hc + 1])
                hxT.append(tx)

            hy_half(0, 0)
            hy_half(1, 0)
            hx_one(0)
            hx_one(1)
            hy_half(0, 1)
            hy_half(1, 1)
            hy_half(2, 0)
            hy_half(3, 0)
            hy_half(2, 1)
            hy_half(3, 1)
            hx_one(2)
            hx_one(3)

            # W3 broadcast to all 128 partitions via a K=1 ones-matmul
            # (gpsimd partition_broadcast would wake the SWDGE ring; a
            # stride-0 DMA would re-read the 2KB row 128 times from HBM).
            ones_col = cpool.tile([1, 128], F32, name="ones_col")
            nc.gpsimd.memset(ones_col[:], 1.0)
            psb = ppool.tile([128, H], F32, name="psb", tag="ps")
            nc.tensor.matmul(psb[:], ones_col[:], w3row[:], start=True, stop=True)
            w3b = cpool.tile([128, H], F32, name="w3b")
            nc.vector.tensor_copy(out=w3b[:], in_=psb[:])

            # PE keepalive over the row-0 stage-A bubble (ACT needs ~700ns
            # before the first real layer-2 matmul can start): keep the HAM
            # activity timer running so the clock ramp isn't reset.
            for _ in range(14):
                nc.tensor.matmul(
                    pswarm[:, 0:64], warm_src[:], warm_src[:, 0:64], start=True, stop=True
                )

            # scores accumulated transposed: scoresT[jc][j, i]
            scoresT = [cpool.tile([128, ROWS], F32, name=f"scT_{jc}") for jc in range(HC)]

            # ---------------- main loop over x rows ----------------
            for i in range(ROWS):
                h1T = []
                for hc in range(HC):
                    # ACT: relu(hyT + hx_i). All of stage A lives on ACT so the
                    # DVE has headroom for the stage-C fused reduce.
                    t = wpool.tile([128, B], F16, name="h1T", tag="h1T", bufs=12)
                    nc.scalar.activation(
                        t[:], hyT[hc][:], Relu, bias=hxT[hc][:, i : i + 1], scale=1.0
                    )
                    h1T.append(t)
                for jc in range(HC):
                    ps2 = ppool.tile([128, B], F32, name="ps2", tag="ps")
                    for hc in range(HC):
                        nc.tensor.matmul(
                            ps2[:],
                            h1T[hc][:, jc * 128 : (jc + 1) * 128],
                            w2sb[hc][:],
                            start=(hc == 0),
                            stop=(hc == HC - 1 and not with_b2),
                        )
                    if with_b2:
                        nc.tensor.matmul(
                            ps2[:], ones1[:], b2row[:], start=False, stop=True
                        )
                    # DVE: scr = relu(ps2) * W3_bcast; scoresT col = sum_k scr
                    scr = wpool.tile([128, B], F32, name="scr", tag="scr", bufs=6)
                    nc.vector.scalar_tensor_tensor(
                        out=scr[:],
                        in0=ps2[:],
                        scalar=0.0,
                        in1=w3b[:],
                        op0=Alu.max,
                        op1=Alu.mult,
                        accum_out=scoresT[jc][:, i : i + 1],
                    )

                # stream the output: every 16 rows, DMA the finished 16-col
                # slab of each scoresT chunk (8KB apiece), so only the last
                # slab remains after the final matmul. Sync ring ONLY: a
                # mid-loop DMA on the scalar ring blocks the ACT engine's
                # strict-FIFO queue on the stt semaphore and starves stage A
                # (measured +43ns on every matmul).
                if i % 16 == 15:
                    lo, hi = i - 15, i + 1
                    for jc in range(HC):
                        nc.sync.dma_start(
                            out=out_d[jc, :, lo:hi], in_=scoresT[jc][:, lo:hi]
                        )

    nc.finalize()  # runs the Bacc pass pipeline (wait splitting etc.)
    return nc


def _get_nc(with_b2: bool):
    if with_b2 not in _BUILT:
        _BUILT[with_b2] = _build(with_b2)
    return _BUILT[with_b2]


def _run(inputs: dict, trace: bool = False, **spmd_kwargs):
    """Shard, execute on 8 cores, gather. Returns (scores, BassKernelResults)."""
    from concourse.bass_utils import run_bass_kernel_spmd

    x = np.asarray(inputs["x"], dtype=np.float32)
    y = np.asarray(inputs["y"], dtype=np.float32)
    W1 = np.asarray(inputs["W1"], dtype=np.float32)
    b1 = np.ascontiguousarray(np.asarray(inputs["b1"], dtype=np.float32))
    W2 = np.asarray(inputs["W2"], dtype=np.float32)
    b2 = np.ascontiguousarray(np.asarray(inputs.get("b2", np.zeros(H)), dtype=np.float32))
    W3 = np.ascontiguousarray(np.asarray(inputs["W3"], dtype=np.float32).reshape(H, 1))
    b3 = np.asarray(inputs.get("b3", np.zeros(1)), dtype=np.float32)

    with_b2 = bool(np.any(b2))
    nc = _get_nc(with_b2)

    W1h = np.ascontiguousarray(W1.astype(np.float16))
    W2h = np.ascontiguousarray(W2.astype(np.float16))
    yTh = np.ascontiguousarray(y.T.astype(np.float16))
    xh = x.astype(np.float16)
    in_maps = []
    for c in range(N_CORES):
        m = {
            "xT": np.ascontiguousarray(xh[c * ROWS : (c + 1) * ROWS].T),
            "yT": yTh,
            "W1": W1h,
            "b1": b1,
            "W2": W2h,
            "W3": W3,
        }
        if with_b2:
            m["b2"] = np.ascontiguousarray(b2.astype(np.float16))
        in_maps.append(m)

    res = run_bass_kernel_spmd(
        nc, in_maps, core_ids=list(range(N_CORES)), trace=trace, **spmd_kwargs
    )
    # outT[jc, j, i] -> scores_slab[i, jc*128 + j]
    slabs = [
        np.transpose(r["outT"], (2, 0, 1)).reshape(ROWS, B) for r in res.results
    ]
    out = np.concatenate(slabs, axis=0)
    if b3.size and np.any(b3):
        out = out + b3.reshape(-1)[0]
    return np.ascontiguousarray(out.astype(np.float32)), res


def kernel(**inputs) -> np.ndarray:
    out, _ = _run(inputs)
    return out



# revision 12
# speedup vs baseline: 1.0074x; 1.0059x over previous
"""ConcatCritic pair-MLP kernel for 8 Trainium2 NeuronCores.

scores[i, j] = MLP(concat(x_i, y_j)) with
MLP = Linear(256,512) -> ReLU -> Linear(512,512) -> ReLU -> Linear(512,1).

Sharding: pure data parallelism over the B^2 pair grid, split along the
x (row) index — each core gets 64 rows of x plus all of y and the full
(small) weight set, and produces a [64, 512] slab of the output.

The host passes x and y pre-transposed (xT [128,64] slab, yT [128,512]) and
receives the output in transposed chunk layout [4][128 j][64 i]; both
transposes are trivial numpy work and remove every on-device transpose.

All matmul operands are fp16 (host-converted): the PE runs fp16 at the same
1 cycle/row as fp32r, but fp16 stationaries re-enable the compiler's fast
weight load (FWL — disabled for fp32/fp32r), hiding the per-matmul LDWEIGHTS
that cost the fp32r version ~22ns/matmul, and input DMA bytes halve.
Accumulation stays fp32 in PSUM; stage C (relu * W3 + accumulate) runs in
fp32 on the DVE, so the only precision loss is fp16 operand rounding
(measured 5e-4 rel-to-max vs the fp32 reference, gate is 2e-2).

Per-core dataflow (per x-row i):
  stage A (ACT): h1T[h, j] = relu(hyT[h, j] + (hx_i[h] + b1[h]))     4x [128,512]
  stage B (PE):  x2[j, k]  = h1_i @ W2   (fp16 matmuls, psum accum)  16x mm
  stage C (DVE): s_i[j]    = sum_k relu(x2[j, k]) * W3[k]            4x scalar_tensor_tensor
with hxT = (x @ W1[:128]).T + b1 and hyT = (y @ W1[128:]).T computed once at
setup. b3 is applied on the host; b2 (nonzero only) via an extra K=1 matmul.
"""

import numpy as np

B = 512
DX = 128
H = 512
N_CORES = 8
ROWS = B // N_CORES  # 64 x-rows per core
HC = H // 128  # 4 chunks of the hidden dim

_BUILT = {}  # with_b2 -> bass.Bass


def _build(with_b2: bool):
    import concourse.mybir as mybir
    from concourse.bacc import Bacc
    from concourse.tile import TileContext

    F32 = mybir.dt.float32
    F16 = mybir.dt.float16
    Relu = mybir.ActivationFunctionType.Relu
    Alu = mybir.AluOpType

    # Bacc (not raw Bass): its compile pipeline splits multi-semaphore waits
    # into event-semaphore chains — TRN2 engine instructions accept only one
    # sync wait, which walrus otherwise rejects.
    nc = Bacc()
    xT_d = nc.declare_dram_parameter("xT", [DX, ROWS], F16, isOutput=False)
    yT_d = nc.declare_dram_parameter("yT", [DX, B], F16, isOutput=False)
    w1_d = nc.declare_dram_parameter("W1", [2 * DX, H], F16, isOutput=False)
    b1_d = nc.declare_dram_parameter("b1", [H], F32, isOutput=False)
    w2_d = nc.declare_dram_parameter("W2", [H, H], F16, isOutput=False)
    w3_d = nc.declare_dram_parameter("W3", [H, 1], F32, isOutput=False)
    if with_b2:
        b2_d = nc.declare_dram_parameter("b2", [H], F16, isOutput=False)
    out_d = nc.declare_dram_parameter("outT", [HC, 128, ROWS], F32, isOutput=True)

    with TileContext(nc) as tc:
        with (
            tc.tile_pool(name="consts", bufs=1) as cpool,
            tc.tile_pool(name="work", bufs=2) as wpool,
            tc.tile_pool(name="psum", bufs=7, space="PSUM") as ppool,
        ):
            # ---------------- input DMAs ----------------
            # Only the two HWDGE rings (sync/SP and scalar/ACT) — the gpsimd
            # SWDGE ring has a slow software drain in the epilogue. The DMA
            # critical path gates hy/hx, which gate everything: tiny tensors
            # (b1, xT) go first for latency, then yT/w1y/w1x each SPLIT in
            # half across both rings so both land ~0.5us earlier, then the W2
            # chunks (first needed ~1.5us later), then w3row (stage C only).
            yT = cpool.tile([DX, B], F16, name="yT")
            w1y = cpool.tile([DX, H], F16, name="w1y")
            w1x = cpool.tile([DX, H], F16, name="w1x")
            b1sb = cpool.tile([128, HC], F32, name="b1sb")  # [p, hc]
            xT = cpool.tile([DX, ROWS], F16, name="xT")
            HB = B // 2
            w3row = cpool.tile([1, H], F32, name="w3row")
            # sync ring: b1+xT (tiny, latency-bound), then the m0 halves of
            # the layer-1 tensors, then W2c0. scalar ring: w3row first (2KB,
            # gates w3b -> the first stts -> psum recycling), then the m1
            # halves, then W2c1. gpsimd SWDGE ring (idle at startup): W2c2/3.
            nc.sync.dma_start(out=b1sb[:], in_=b1_d[:].rearrange("(c p) -> p c", p=128))
            nc.sync.dma_start(out=xT[:], in_=xT_d[:, :])
            nc.scalar.dma_start(out=w3row[:], in_=w3_d[:, :].flatten().unsqueeze(0))
            nc.sync.dma_start(out=yT[:, 0:HB], in_=yT_d[:, 0:HB])
            nc.scalar.dma_start(out=yT[:, HB:], in_=yT_d[:, HB:])
            nc.sync.dma_start(out=w1y[:, 0:HB], in_=w1_d[DX : 2 * DX, 0:HB])
            nc.scalar.dma_start(out=w1y[:, HB:], in_=w1_d[DX : 2 * DX, HB:])
            nc.sync.dma_start(out=w1x[:, 0:HB], in_=w1_d[0:DX, 0:HB])
            nc.scalar.dma_start(out=w1x[:, HB:], in_=w1_d[0:DX, HB:])
            w2sb = [cpool.tile([128, H], F16, name=f"w2_{hc}") for hc in range(HC)]
            w2r = w2_d[:, :].rearrange("(c p) k -> p c k", p=128)
            nc.gpsimd.dma_start(out=w2sb[2][:], in_=w2r[:, 2, :])
            nc.gpsimd.dma_start(out=w2sb[3][:], in_=w2r[:, 3, :])
            nc.sync.dma_start(out=w2sb[0][:], in_=w2r[:, 0, :])
            nc.scalar.dma_start(out=w2sb[1][:], in_=w2r[:, 1, :])
            if with_b2:
                b2row = cpool.tile([1, H], F16, name="b2row")
                nc.scalar.dma_start(out=b2row[:], in_=b2_d[:].unsqueeze(0))
                ones1 = cpool.tile([1, 128], F16, name="ones1")
                nc.vector.memset(ones1[:], 1.0)

            # PE warmup. Measured across many traces: the HAM clock ramp to
            # 2.4GHz completes ~13us after the FIRST PE instruction, largely
            # independent of gaps or matmul width — so start the PE as early
            # as possible (warm_src memset on the DVE, which is idle ~1us
            # before gpsimd gets there) and bridge with cheap N=64 matmuls
            # until the layer-1 DMAs land.
            warm_src = cpool.tile([1, 128], F16, name="warm_src")
            nc.vector.memset(warm_src[:], 1.0)
            pswarm = ppool.tile([128, B], F32, name="pswarm", tag="warm", bufs=1)
            for _ in range(52):
                nc.tensor.matmul(
                    pswarm[:, 0:64], warm_src[:], warm_src[:, 0:64], start=True, stop=True
                )

            # hy in half-width (N=256) matmuls against the yT halves, and
            # interleaved with hx, so PE work is gated on whichever DMA ring
            # happens to land first rather than on all of yT+w1y; hc 0/1
            # stationaries live in the m0 halves (sync ring), hc 2/3 in m1
            # (scalar ring). hyT[hc][h, j] = (y @ W1y).T stored fp16;
            # hxT[hc][h, i] = (x @ W1x).T + b1 kept fp32.
            hyT = [cpool.tile([128, B], F16, name=f"hyT_{hc}") for hc in range(HC)]
            hxT = []
            pshy = [None] * HC

            def hy_half(hc, m):
                lo, hi = m * HB, (m + 1) * HB
                if pshy[hc] is None:
                    pshy[hc] = ppool.tile([128, B], F32, name="pshy", tag="ps")
                nc.tensor.matmul(
                    pshy[hc][:, lo:hi],
                    w1y[:, hc * 128 : (hc + 1) * 128],
                    yT[:, lo:hi],
                    start=True,
                    stop=True,
                )
                nc.vector.tensor_copy(out=hyT[hc][:, lo:hi], in_=pshy[hc][:, lo:hi])

            def hx_one(hc):
                pshx = ppool.tile([128, ROWS], F32, name="pshx", tag="ps")
                nc.tensor.matmul(
                    pshx[:], w1x[:, hc * 128 : (hc + 1) * 128], xT[:], start=True, stop=True
                )
                tx = cpool.tile([128, ROWS], F32, name=f"hxT_{hc}")
                nc.vector.tensor_scalar_add(tx[:], pshx[:], b1sb[:, hc : hc + 1])
                hxT.append(tx)

            hy_half(0, 0)
            hy_half(1, 0)
            hx_one(0)
            hx_one(1)
            hy_half(0, 1)
            hy_half(1, 1)
            hy_half(2, 0)
            hy_half(3, 0)
            hy_half(2, 1)
            hy_half(3, 1)
            hx_one(2)
            hx_one(3)

            # W3 broadcast to all 128 partitions via a K=1 ones-matmul
            # (gpsimd partition_broadcast would wake the SWDGE ring; a
            # stride-0 DMA would re-read the 2KB row 128 times from HBM).
            ones_col = cpool.tile([1, 128], F32, name="ones_col")
            nc.gpsimd.memset(ones_col[:], 1.0)
            psb = ppool.tile([128, H], F32, name="psb", tag="ps")
            nc.tensor.matmul(psb[:], ones_col[:], w3row[:], start=True, stop=True)
            w3b = cpool.tile([128, H], F32, name="w3b")
            nc.vector.tensor_copy(out=w3b[:], in_=psb[:])

            # PE keepalive over the row-0 stage-A bubble (ACT needs ~700ns
            # before the first real layer-2 matmul can start): keep the HAM
            # activity timer running so the clock ramp isn't reset.
            for _ in range(14):
                nc.tensor.matmul(
                    pswarm[:, 0:64], warm_src[:], warm_src[:, 0:64], start=True, stop=True
                )

            # scores accumulated transposed: scoresT[jc][j, i]
            scoresT = [cpool.tile([128, ROWS], F32, name=f"scT_{jc}") for jc in range(HC)]

            # ---------------- main loop over x rows ----------------
            for i in range(ROWS):
                h1T = []
                for hc in range(HC):
                    # ACT: relu(hyT + hx_i). All of stage A lives on ACT so the
                    # DVE has headroom for the stage-C fused reduce.
                    t = wpool.tile([128, B], F16, name="h1T", tag="h1T", bufs=12)
                    nc.scalar.activation(
                        t[:], hyT[hc][:], Relu, bias=hxT[hc][:, i : i + 1], scale=1.0
                    )
                    h1T.append(t)
                for jc in range(HC):
                    ps2 = ppool.tile([128, B], F32, name="ps2", tag="ps")
                    for hc in range(HC):
                        nc.tensor.matmul(
                            ps2[:],
                            h1T[hc][:, jc * 128 : (jc + 1) * 128],
                            w2sb[hc][:],
                            start=(hc == 0),
                            stop=(hc == HC - 1 and not with_b2),
                        )
                    if with_b2:
                        nc.tensor.matmul(
                            ps2[:], ones1[:], b2row[:], start=False, stop=True
                        )
                    # DVE: scr = relu(ps2) * W3_bcast; scoresT col = sum_k scr
                    scr = wpool.tile([128, B], F32, name="scr", tag="scr", bufs=6)
                    nc.vector.scalar_tensor_tensor(
                        out=scr[:],
                        in0=ps2[:],
                        scalar=0.0,
                        in1=w3b[:],
                        op0=Alu.max,
                        op1=Alu.mult,
                        accum_out=scoresT[jc][:, i : i + 1],
                    )

                # stream the output: every 16 rows, DMA the finished 16-col
                # slab of each scoresT chunk (8KB apiece), so only the last
                # slab remains after the final matmul. Sync ring ONLY: a
                # mid-loop DMA on the scalar ring blocks the ACT engine's
                # strict-FIFO queue on the stt semaphore and starves stage A
                # (measured +43ns on every matmul).
                if i % 16 == 15:
                    lo, hi = i - 15, i + 1
                    for jc in range(HC):
                        nc.sync.dma_start(
                            out=out_d[jc, :, lo:hi], in_=scoresT[jc][:, lo:hi]
                        )

    nc.finalize()  # runs the Bacc pass pipeline (wait splitting etc.)
    return nc


def _get_nc(with_b2: bool):
    if with_b2 not in _BUILT:
        _BUILT[with_b2] = _build(with_b2)
    return _BUILT[with_b2]


def _run(inputs: dict, trace: bool = False, **spmd_kwargs):
    """Shard, execute on 8 cores, gather. Returns (scores, BassKernelResults)."""
    from concourse.bass_utils import run_bass_kernel_spmd

    x = np.asarray(inputs["x"], dtype=np.float32)
    y = np.asarray(inputs["y"], dtype=np.float32)
    W1 = np.asarray(inputs["W1"], dtype=np.float32)
    b1 = np.ascontiguousarray(np.asarray(inputs["b1"], dtype=np.float32))
    W2 = np.asarray(inputs["W2"], dtype=np.float32)
    b2 = np.ascontiguousarray(np.asarray(inputs.get("b2", np.zeros(H)), dtype=np.float32))
    W3 = np.ascontiguousarray(np.asarray(inputs["W3"], dtype=np.float32).reshape(H, 1))
    b3 = np.asarray(inputs.get("b3", np.zeros(1)), dtype=np.float32)

    with_b2 = bool(np.any(b2))
    nc = _get_nc(with_b2)

    W1h = np.ascontiguousarray(W1.astype(np.float16))
    W2h = np.ascontiguousarray(W2.astype(np.float16))
    yTh = np.ascontiguousarray(y.T.astype(np.float16))
    xh = x.astype(np.float16)
    in_maps = []
    for c in range(N_CORES):
        m = {
            "xT": np.ascontiguousarray(xh[c * ROWS : (c + 1) * ROWS].T),
            "yT": yTh,
            "W1": W1h,
            "b1": b1,
            "W2": W2h,
            "W3": W3,
        }
        if with_b2:
            m["b2"] = np.ascontiguousarray(b2.astype(np.float16))
        in_maps.append(m)

    res = run_bass_kernel_spmd(
        nc, in_maps, core_ids=list(range(N_CORES)), trace=trace, **spmd_kwargs
    )
    # outT[jc, j, i] -> scores_slab[i, jc*128 + j]
    slabs = [
        np.transpose(r["outT"], (2, 0, 1)).reshape(ROWS, B) for r in res.results
    ]
    out = np.concatenate(slabs, axis=0)
    if b3.size and np.any(b3):
        out = out + b3.reshape(-1)[0]
    return np.ascontiguousarray(out.astype(np.float32)), res


def kernel(**inputs) -> np.ndarray:
    out, _ = _run(inputs)
    return out

